# revision 16
# baseline (speedup 1.0000x reference)
"""CARAFE + MSGConv Trainium2 kernel (8 NeuronCores, spatial x batch sharding).

Decomposition notes
-------------------
out[c, i, j] = sum_{p,q} W[5p+q, i, j] * Xpad[c, i//2 + p - 2, j//2 + q - 2]
 (taps live at source resolution; identical for both subpixel parities).

Per core: one batch element (core//4) and a 16-source-row block (core%4).
The 25-tap reassembly runs on the TensorEngine as one K=120 matmul per
(row-pair, column-quarter) block:
  out[c, n] = sum_{(u,v)} X6T[(u,v), c] * B4[(u,v), n]
where B4 is a banded matrix of softmaxed W values, built at runtime with
gpsimd local_scatter (per-partition index scatter) + a PE transpose.
"""

import sys

sys.path.insert(0, "/opt/trn_rl_repo")

from contextlib import ExitStack

import ml_dtypes
import numpy as np

import concourse.bass as bass
import concourse.tile as tile
from concourse import bacc, library_config, mybir
from concourse.bass_utils import run_bass_kernel_spmd

BF16 = mybir.dt.bfloat16
F32 = mybir.dt.float32
I16 = mybir.dt.int16
AF = mybir.ActivationFunctionType
OP = mybir.AluOpType
nbf = ml_dtypes.bfloat16

# ---- geometry ----
C = 128
H = W = 64
NCORES = 8
RB = 16          # source rows handled per core
XR = 24          # X shard rows (RB + 4 halo each side)
XW = 68          # padded width (64 + 2 each side)
NEG = -30.0      # additive pre-activation mask; SiLU(-30) ~= -2.8e-12


# ======================================================================
# host-side parameter prep
# ======================================================================

def _fold_1x1(w, s):
    # w [cout, cin, 1, 1] -> lhsT [cin, cout], BN scale folded
    return (w[:, :, 0, 0] * s[:, None]).T.copy()


def _dw_taps(w, s, k):
    # w [ch, 1, k, k] -> [ch, 25] unified 5x5 tap grid (zeros outside kxk)
    ch = w.shape[0]
    out = np.zeros((ch, 25), np.float32)
    off = (5 - k) // 2
    for ty in range(k):
        for tx in range(k):
            out[:, 5 * (ty + off) + (tx + off)] = w[:, 0, ty, tx] * s
    return out


def _host_consts(inputs):
    d = {}
    d["w_cv1"] = _fold_1x1(inputs["comp_cv1_w"], inputs["comp_cv1_s"]).astype(nbf)
    d["b_cv1"] = inputs["comp_cv1_b"].reshape(32, 1).astype(np.float32)
    w3 = _dw_taps(inputs["comp_dw3_w"], inputs["comp_dw3_s"], 3)
    w5 = _dw_taps(inputs["comp_dw5_w"], inputs["comp_dw5_s"], 5)
    wdw = np.concatenate([w3, w5], 0)                     # [32, 25]
    d["w_dwp"] = np.tile(wdw, (4, 1)).astype(np.float32)         # [128, 25]
    bdw = np.concatenate([inputs["comp_dw3_b"], inputs["comp_dw5_b"]])
    d["b_dwp"] = np.tile(bdw, 4).reshape(128, 1).astype(np.float32)
    d["w_px"] = _fold_1x1(inputs["comp_px_w"], inputs["comp_px_s"]).astype(nbf)
    d["b_px"] = inputs["comp_px_b"].reshape(64, 1).astype(np.float32)

    we = _fold_1x1(inputs["enc_cv1_w"], inputs["enc_cv1_s"])  # [64, 50]
    d["w_ecv1"] = np.concatenate([we, np.ones((1, 50), np.float32)], 0).astype(nbf)
    d["b_ecv1"] = inputs["enc_cv1_b"].reshape(50, 1).astype(np.float32)
    e3 = _dw_taps(inputs["enc_dw3_w"], inputs["enc_dw3_s"], 3)
    e5 = _dw_taps(inputs["enc_dw5_w"], inputs["enc_dw5_s"], 5)
    wedw = np.concatenate([e3, e5], 0)                    # [50, 25]
    d["w_edwp"] = np.tile(wedw, (2, 1)).astype(np.float32)       # [100, 25]
    bedw = np.concatenate([inputs["enc_dw3_b"], inputs["enc_dw5_b"]])
    d["b_edwp"] = np.tile(bedw, 2).reshape(100, 1).astype(np.float32)
    wpx = _fold_1x1(inputs["enc_px_w"], inputs["enc_px_s"])  # [100, 100]
    d["w_epx"] = np.concatenate(
        [wpx, inputs["enc_px_b"].reshape(1, 100)], 0
    ).astype(nbf)                                          # [101, 100]

    d["ones1"] = np.ones((1, 32), nbf)
    d["erow1"] = np.ones((1, 20 * XW), nbf)
    d["ident"] = np.eye(128, dtype=nbf)



    # repl [128, 4*128]: lhsT for the W row-replication matmul
    # n is output-raster order within the block: n = 32*(2*yl+dy) + (2*xl+dx)
    rp = np.zeros((128, 512), np.float32)
    for jb in range(4):
        for n in range(128):
            rho, j = divmod(n, 32)
            yl, xl = rho // 2, j // 2
            m = 64 * yl + 16 * jb + xl
            rp[m, 128 * jb + n] = 1.0
    d["repl"] = rp.astype(nbf)

    # sidx [128, 8*100] int16: local_scatter indices (8 blocks per call).
    # n raster within block; out-of-image horizontal taps dropped (-1).
    si = np.full((128, 800), -1, np.int16)
    for n in range(128):
        rho, j = divmod(n, 32)
        yl, dy = divmod(rho, 2)
        xl, dx = divmod(j, 2)
        sn = 2 * dy + dx
        for bb in range(8):
            jb = bb % 4
            for cp in range(100):
                sc, k = divmod(cp, 25)
                if sc != sn:
                    continue
                p, q = divmod(k, 5)
                if not (0 <= 16 * jb + xl + q - 2 < 64):
                    continue
                si[n, 100 * bb + cp] = 120 * bb + 20 * (yl + p) + (xl + q)
    d["sidx"] = si
    return d


def _host_shard(X, core):
    b, ri = divmod(core, 4)
    r0 = 16 * ri - 4
    xs = np.zeros((C, XR, XW), np.float32)
    lo, hi = max(0, r0), min(H, r0 + XR)
    xs[:, lo - r0 : hi - r0, 2 : 2 + W] = X[b, :, lo:hi, :]
    mrow = np.zeros((1, XR, XW), np.float32)
    for r in range(XR):
        if not (0 <= r0 + r < H):
            mrow[0, r, 2 : 2 + W] = NEG
    emask = np.zeros((1, 20, XW), np.float32)
    for r in range(20):
        if not (0 <= (16 * ri - 2) + r < H):
            emask[0, r, 2 : 2 + W] = NEG
    xsb = xs.astype(nbf)
    xt = np.zeros((120, 32 * 128), nbf)
    for B in range(32):
        t, jb = divmod(B, 4)
        slab = xsb[:, 2 * t + 2 : 2 * t + 8, 16 * jb : 16 * jb + 20]  # [c,6,20]
        xt[:, 128 * B : 128 * B + 128] = slab.reshape(C, 120).T
    return (
        xsb.reshape(C, XR * XW),
        mrow.reshape(1, XR * XW).astype(nbf),
        emask.reshape(1, 20 * XW).astype(nbf),
        xt,
    )


# ======================================================================
# device kernel
# ======================================================================

def build_kernel():
    nc = bacc.Bacc(
        "TRN2",
        target_bir_lowering=False,
        debug=False,
        enable_asserts=False,
        num_devices=NCORES,
    )

    def din(name, shape, dt):
        return nc.dram_tensor(name, list(shape), dt, kind="ExternalInput").ap()

    x_d = din("x", (128, XR * XW), BF16)
    xt_d = din("xt", (120, 32 * 128), BF16)
    mrow_d = din("mrow", (1, XR * XW), BF16)
    emask_d = din("emask", (1, 20 * XW), BF16)
    erow1_d = din("erow1", (1, 20 * XW), BF16)
    w_cv1_d = din("w_cv1", (128, 32), BF16)
    b_cv1_d = din("b_cv1", (32, 1), F32)
    ones1_d = din("ones1", (1, 32), BF16)
    w_dwp_d = din("w_dwp", (128, 25), F32)
    b_dwp_d = din("b_dwp", (128, 1), F32)
    w_px_d = din("w_px", (64, 64), BF16)
    b_px_d = din("b_px", (64, 1), F32)
    w_ecv1_d = din("w_ecv1", (65, 50), BF16)
    b_ecv1_d = din("b_ecv1", (50, 1), F32)
    w_edwp_d = din("w_edwp", (100, 25), F32)
    b_edwp_d = din("b_edwp", (100, 1), F32)
    w_epx_d = din("w_epx", (101, 100), BF16)
    repl_d = din("repl", (128, 512), BF16)
    ident_d = din("ident", (128, 128), BF16)
    sidx_d = din("sidx", (128, 800), I16)
    out_d = nc.dram_tensor("out", [128, 32 * 128], F32, kind="ExternalOutput").ap()
    out3 = out_d.rearrange("c (r j) -> c r j", j=128)

    with tile.TileContext(nc) as tc, ExitStack() as ctx:
        cpool = ctx.enter_context(tc.tile_pool(name="consts", bufs=1))
        work = ctx.enter_context(tc.tile_pool(name="work", bufs=1))
        psA = ctx.enter_context(tc.tile_pool(name="psA", bufs=2, space="PSUM"))
        psB = ctx.enter_context(tc.tile_pool(name="psB", bufs=2, space="PSUM"))
        psO = ctx.enter_context(tc.tile_pool(name="psO", bufs=2, space="PSUM"))
        spool = ctx.enter_context(tc.tile_pool(name="stage", bufs=3))

        nc.gpsimd.load_library(library_config.local_scatter)

        def cload(ap_d, shape, dt, eng=None):
            t = cpool.tile(list(shape), dt, tag=ap_d.tensor.name)
            (eng or nc.sync).dma_start(t[:], ap_d)
            return t

        mrow = cload(mrow_d, (1, XR * XW), BF16)
        w_cv1 = cload(w_cv1_d, (128, 32), BF16)
        b_cv1 = cload(b_cv1_d, (32, 1), F32)
        ones1 = cload(ones1_d, (1, 32), BF16)
        w_dwp = cload(w_dwp_d, (128, 25), F32)
        b_dwp = cload(b_dwp_d, (128, 1), F32)
        w_px = cload(w_px_d, (64, 64), BF16)
        b_px = cload(b_px_d, (64, 1), F32)
        w_ecv1 = cload(w_ecv1_d, (65, 50), BF16)
        b_ecv1 = cload(b_ecv1_d, (50, 1), F32)
        w_edwp = cload(w_edwp_d, (100, 25), F32)
        b_edwp = cload(b_edwp_d, (100, 1), F32)
        w_epx = cload(w_epx_d, (101, 100), BF16)
        ident = cload(ident_d, (128, 128), BF16, eng=nc.scalar)
        xb = cpool.tile([128, XR * XW], BF16, tag="x")
        for ch in range(3):
            nc.sync.dma_start(
                xb[:, 8 * XW * ch : 8 * XW * (ch + 1)],
                x_d[:, 8 * XW * ch : 8 * XW * (ch + 1)],
            )
        xt = cpool.tile([120, 32 * 128], BF16, tag="xt")
        for ch in range(2):
            nc.scalar.dma_start(
                xt[:, 2048 * ch : 2048 * (ch + 1)],
                xt_d[:, 2048 * ch : 2048 * (ch + 1)],
            )
        repl = cload(repl_d, (128, 512), BF16, eng=nc.scalar)
        sidx = cload(sidx_d, (128, 800), I16, eng=nc.scalar)

        # warmup: trigger the local_scatter ucode library load early so it
        # overlaps the conv front instead of stalling the first real scatter
        warm = work.tile([16, 16], BF16)
        nc.gpsimd.local_scatter(
            warm[:], ident[0:16, 0:2], sidx[0:16, 0:2],
            channels=16, num_elems=16, num_idxs=2,
        )

        xb3 = xb[:].rearrange("p (r c) -> p r c", c=XW)
        mrow3 = mrow[:].rearrange("p (r c) -> p r c", c=XW)

        # persistent working tensors
        x12 = work.tile([64, XR * XW], BF16)       # comp x1 (0:32) + x2 (32:64)
        enc_in = work.tile([65, 20 * XW], BF16)    # px out + mask row
        enc_cat = work.tile([101, 16 * W], BF16)   # enc x1/x2 + ones (64-wide)
        e1c = work.tile([50, 20 * W], BF16)        # enc cv1 out, 64-wide, 20 rows
        x1p = work.tile([128, 9 * XW], BF16)       # packed x1 for dw
        e1p = work.tile([100, 12 * XW], BF16)      # packed enc x1 for dw
        ET = work.tile([128, 800], F32)            # enc-px outputs (8 row-pairs)
        expv = work.tile([128, 800], F32)          # exp, s-major [s][t][k]
        S = work.tile([128, 32], F32)
        R = work.tile([128, 32], F32)
        wcat = work.tile([128, 800], BF16)         # softmaxed W, [t][s][k]
        dall = work.tile([128, 3200], BF16)        # replicated W rows per block

        x12_3 = x12[:].rearrange("p (r c) -> p r c", c=XW)
        enc_in3 = enc_in[:].rearrange("p (r c) -> p r c", c=XW)
        enc_cat3 = enc_cat[:].rearrange("p (r c) -> p r c", c=W)
        e1c3 = e1c[:].rearrange("p (r c) -> p r c", c=W)
        x1p3 = x1p[:].rearrange("p (r c) -> p r c", c=XW)
        e1p3 = e1p[:].rearrange("p (r c) -> p r c", c=XW)

        nc.vector.memset(x12[:], 0.0)
        nc.vector.memset(enc_in[:], 0.0)
        nc.sync.dma_start(enc_cat[100:101, :], erow1_d[:, 0 : 16 * W])
        nc.sync.dma_start(enc_in[64:65, :], emask_d)

        # ---- comp cv1: 1x1 conv 128->32 (+ SiLU, + out-of-image row mask)
        for ch in range(3):
            r0 = 8 * ch
            ps = psA.tile([32, 512], F32, tag="convps")
            nc.tensor.matmul(
                ps[:], w_cv1[:], xb3[0:128, r0 : r0 + 8, 2 : 2 + W],
                start=True, stop=False,
            )
            nc.tensor.matmul(
                ps[:], ones1[:], mrow3[0:1, r0 : r0 + 8, 2 : 2 + W],
                start=False, stop=True,
            )
            nc.scalar.activation(
                x12_3[0:32, r0 : r0 + 8, 2 : 2 + W],
                ps[:].rearrange("p (r c) -> p r c", c=W),
                AF.Silu, bias=b_cv1[:, 0:1],
            )

        # ---- comp dw3/dw5 (unified 5x5 taps, rows packed 4x32)
        for g in range(4):
            nc.sync.dma_start(
                x1p[32 * g : 32 * g + 32, :],
                x12[0:32, 5 * g * XW : (5 * g + 9) * XW],
            )
        acc_a = work.tile([128, 5 * W], BF16)
        acc_b = work.tile([128, 5 * W], BF16)
        accv = [acc_a[:].rearrange("p (r c) -> p r c", c=W),
                acc_b[:].rearrange("p (r c) -> p r c", c=W)]
        for t in range(25):
            ty, tx = divmod(t, 5)
            src = x1p3[:, ty : ty + 5, tx : tx + W]
            av = accv[t % 2]
            if t < 2:
                nc.vector.tensor_scalar(
                    av, src, w_dwp[:, t : t + 1], None, OP.mult
                )
            else:
                nc.vector.scalar_tensor_tensor(
                    av, src, w_dwp[:, t : t + 1], av, OP.mult, OP.add
                )
        nc.vector.tensor_add(accv[0], accv[0], accv[1])
        x2p = work.tile([128, 5 * W], BF16)
        nc.scalar.activation(x2p[:], acc_a[:], AF.Silu, bias=b_dwp[:, 0:1])
        for g in range(4):
            nc.sync.dma_start(
                x12_3[32:64, 2 + 5 * g : 7 + 5 * g, 2 : 2 + W],
                x2p[32 * g : 32 * g + 32, :].rearrange("p (r c) -> p r c", c=W),
            )

        # ---- comp px: 1x1 conv 64->64 (+ SiLU)
        for ch, (r0, nr) in enumerate(((0, 8), (8, 8), (16, 4))):
            ps = psA.tile([64, 512], F32, tag="convps")
            nc.tensor.matmul(
                ps[:, : nr * W],
                w_px[:],
                x12_3[0:64, 2 + r0 : 2 + r0 + nr, 2 : 2 + W],
                start=True, stop=True,
            )
            nc.scalar.activation(
                enc_in3[0:64, r0 : r0 + nr, 2 : 2 + W],
                ps[:, : nr * W].rearrange("p (r c) -> p r c", c=W),
                AF.Silu, bias=b_px[:, 0:1],
            )

        # ---- enc cv1: 1x1 conv 64->50 (+ SiLU, mask row rides K=65)
        for ch, (r0, nr) in enumerate(((0, 8), (8, 8), (16, 4))):
            ps = psA.tile([50, 512], F32, tag="convps")
            nc.tensor.matmul(
                ps[:, : nr * W],
                w_ecv1[:],
                enc_in3[0:65, r0 : r0 + nr, 2 : 2 + W],
                start=True, stop=True,
            )
            nc.scalar.activation(
                e1c3[0:50, r0 : r0 + nr, :],
                ps[:, : nr * W].rearrange("p (r c) -> p r c", c=W),
                AF.Silu, bias=b_ecv1[:, 0:1],
            )

        # ---- enc dw3/dw5 (rows packed 2x50)
        nc.vector.memset(e1p[:], 0.0)
        for g in range(2):
            nc.sync.dma_start(
                e1p3[50 * g : 50 * g + 50, 0:12, 2 : 2 + W],
                e1c3[0:50, 8 * g : 8 * g + 12, :],
            )
        nc.sync.dma_start(
            enc_cat[0:50, :], e1c3[0:50, 2:18, :]
        )
        acc2_a = work.tile([100, 8 * W], BF16)
        acc2_b = work.tile([100, 8 * W], BF16)
        acc2v = [acc2_a[:].rearrange("p (r c) -> p r c", c=W),
                 acc2_b[:].rearrange("p (r c) -> p r c", c=W)]
        for t in range(25):
            ty, tx = divmod(t, 5)
            src = e1p3[:, ty : ty + 8, tx : tx + W]
            av = acc2v[t % 2]
            if t < 2:
                nc.vector.tensor_scalar(
                    av, src, w_edwp[:, t : t + 1], None, OP.mult
                )
            else:
                nc.vector.scalar_tensor_tensor(
                    av, src, w_edwp[:, t : t + 1], av, OP.mult, OP.add
                )
        nc.vector.tensor_add(acc2v[0], acc2v[0], acc2v[1])
        e2p = work.tile([100, 8 * W], BF16)
        nc.scalar.activation(e2p[:], acc2_a[:], AF.Silu, bias=b_edwp[:, 0:1])
        for g in range(2):
            nc.sync.dma_start(
                enc_cat3[50:100, 8 * g : 8 * g + 8, :],
                e2p[50 * g : 50 * g + 50, :].rearrange("p (r c) -> p r c", c=W),
            )

        # ---- enc px (transposed output: M = 128 pixels of a row-pair)
        for t in range(8):
            ps = psA.tile([128, 100], F32, tag="convps")
            nc.tensor.matmul(
                ps[:], enc_cat[0:101, 128 * t : 128 * t + 128],
                w_epx[:], start=True, stop=True,
            )
            nc.scalar.activation(
                ET[:, 100 * t : 100 * t + 100], ps[:], AF.Silu
            )

        # ---- softmax over 25 taps, per subposition s (no max-subtraction)
        ET3 = ET[:].rearrange("p (t e) -> p t e", e=100)
        exp3 = expv[:].rearrange("p (s t k) -> p s t k", s=4, t=8)
        for s in range(4):
            nc.scalar.activation(exp3[:, s], ET3[:, :, s::4], AF.Exp)
            nc.vector.tensor_reduce(
                S[:, 8 * s : 8 * s + 8], exp3[:, s], mybir.AxisListType.X, OP.add
            )
        nc.vector.reciprocal(R[:], S[:])
        for t in range(8):
            for s in range(4):
                nc.vector.tensor_scalar(
                    wcat[:, 100 * t + 25 * s : 100 * t + 25 * s + 25],
                    exp3[:, s, t],
                    R[:, 8 * s + t : 8 * s + t + 1],
                    None,
                    OP.mult,
                )

        # ---- per-block pipeline ----
        # replicate W rows to output-pixel partitions
        for B in range(32):
            t, jb = divmod(B, 4)
            ps = psB.tile([128, 100], F32, tag="small")
            nc.tensor.matmul(
                ps[:],
                repl[:, 128 * jb : 128 * jb + 128],
                wcat[:, 100 * t : 100 * t + 100],
                start=True, stop=True,
            )
            dst = dall[:, 100 * B : 100 * B + 100]
            if B % 2 == 0:
                nc.vector.tensor_copy(dst, ps[:])
            else:
                nc.scalar.copy(dst, ps[:])

        b4t = work.tile([128, 4 * 960], BF16)
        for cl in range(4):
            nc.gpsimd.local_scatter(
                b4t[:, 960 * cl : 960 * cl + 960],
                dall[:, 800 * cl : 800 * cl + 800],
                sidx[:],
                channels=128,
                num_elems=960,
                num_idxs=800,
            )

        for B in range(32):
            t, jb = divmod(B, 4)
            cl, bb = divmod(B, 8)
            psb4 = psB.tile([120, 128], BF16, tag="b4t")
            nc.tensor.transpose(
                psb4[:], b4t[:, 960 * cl + 120 * bb : 960 * cl + 120 * bb + 120], ident[:]
            )
            b4 = spool.tile([120, 128], BF16, tag="b4")
            if B % 2 == 0:
                nc.scalar.copy(b4[:], psb4[:])
            else:
                nc.vector.tensor_copy(b4[:], psb4[:])

            po = psO.tile([128, 128], F32, tag="out")
            nc.tensor.matmul(
                po[:], xt[:, 128 * B : 128 * B + 128], b4[:],
                start=True, stop=True,
            )

            stg = spool.tile([128, 128], F32, tag="ostage")
            if B % 2 == 0:
                nc.vector.tensor_copy(stg[:], po[:])
            else:
                nc.scalar.copy(stg[:], po[:])
            (nc.sync if B % 2 == 0 else nc.scalar).dma_start(
                out3[:, 4 * t : 4 * t + 4, 32 * jb : 32 * jb + 32],
                stg[:].rearrange("c (r j) -> c r j", j=32),
            )

    nc.compile()
    return nc


_NC_CACHE = None


def _get_nc():
    global _NC_CACHE
    if _NC_CACHE is None:
        _NC_CACHE = build_kernel()
    return _NC_CACHE


def kernel(**inputs) -> np.ndarray:
    X = np.asarray(inputs["X"], np.float32)
    consts = _host_consts(
        {k: np.asarray(v, np.float32) for k, v in inputs.items() if k != "X"}
    )
    in_maps = []
    for core in range(NCORES):
        xs, mrow, emask, xt = _host_shard(X, core)
        m = dict(consts)
        m["x"] = xs
        m["mrow"] = mrow
        m["emask"] = emask
        m["xt"] = xt
        in_maps.append(m)

    nc = _get_nc()
    res = run_bass_kernel_spmd(nc, in_maps, core_ids=list(range(NCORES)))
    out = np.zeros((2, C, 128, 128), np.float32)
    for core in range(NCORES):
        b, ri = divmod(core, 4)
        out[b, :, 32 * ri : 32 * ri + 32, :] = (
            res.results[core]["out"].reshape(C, 32, 128)
        )
    return out


if __name__ == "__main__":
    rng = np.random.default_rng(0)
    X = rng.normal(size=(2, C, H, W)).astype(np.float32)
    print("smoke build only")
    build_kernel()
    print("build ok")
